# revision 2
# baseline (speedup 1.0000x reference)
"""Multi-head dot-product attention (per-head LayerNorm on q/k/v) on 8
Trainium2 NeuronCores — overlap-optimized bf16 rewrite.

Sharding: core = (batch, query-half), as the baseline: each core owns one
batch and 1024 query tokens, computes k/v for the full 2048 keys of its
batch (pure SPMD, zero collectives; host rotates tokens per core).

Structure vs the original baseline (all matmuls stay bf16 — fp8 attention
measured 0.023-0.034 rel err, over the 2e-2 gate):
 - projections split into head halves: heads 0-7 project first, then their
   attention (ScalarE exp-bound) runs while heads 8-15 project on PE/DVE;
   out-projection of early query tiles overlaps the remaining attention.
 - per-head projection means for all tiles are computed up front via the
   host-augmented mean columns (psM bank freed before attention pools open).
 - exp instructions span two key tiles ([128, 2, 512] PSUM reads),
   amortizing ScalarE's ~185ns per-instruction overhead.
 - pv runs transposed: probs tiles are the (free) stationary, v the moving
   operand, so each matmul charges 64 rows instead of 512 — attention
   output lands as [query, dim] and the softmax denominators come from
   1-row matmuls against a ones vector, normalized per-partition on DVE
   (no DRAM-bounce broadcast).
 - LayerNorm: centering in the PSUM-drain subtract (DVE); square on GPSIMD
   (Pool); per-head sum-of-squares reduce on DVE in bf16; rsqrt via 2
   Newton iterations; the rstd multiply runs on ScalarE (Copy, per-
   partition scale) in phase 1 while it is idle, on Pool afterwards.
 - xT's SBUF space is released after the last projection and reused for
   Wo and the transposed attention output (aT).
"""

import sys

for _p in ("/opt/trn_rl_repo",):
    if _p not in sys.path:
        sys.path.insert(0, _p)

import numpy as np
import ml_dtypes
from contextlib import ExitStack

import concourse.bass as bass
import concourse.bacc as bacc
import concourse.tile as tile
from concourse import mybir
from concourse import bass_utils

BF16 = ml_dtypes.bfloat16

B, S, DM = 4, 2048, 1024
H, HD = 16, 64
NCORES = 8
SQ = S // 2          # query tokens per core
NT_K = S // 128      # 16 token tiles for k/v
NT_Q = SQ // 128     # 8 token tiles for q
NIT = DM // 128      # 8 contraction tiles
QB = 512             # query block width in attention
NQB = SQ // QB       # 2
HH = H // 2          # heads per half (8)
NQC = QB // 128      # query chunks per block (4)
LN_EPS = 1e-5


def _build_program():
    nc = bacc.Bacc("TRN2", target_bir_lowering=False, debug=False)

    f32 = mybir.dt.float32
    bf16 = mybir.dt.bfloat16
    u16 = mybir.dt.uint16
    i32 = mybir.dt.int32

    xT_d = nc.dram_tensor("xt", [DM + 1, S], bf16, kind="ExternalInput").ap()
    w_d = {
        n: nc.dram_tensor(f"w{n}", [DM + 1, DM + H], bf16, kind="ExternalInput").ap()
        for n in ("q", "k", "v")
    }
    w_d["o"] = nc.dram_tensor("wo", [DM, DM], bf16, kind="ExternalInput").ap()
    bo_d = nc.dram_tensor("bo", [1, DM], f32, kind="ExternalInput").ap()
    out_d = nc.dram_tensor("out", [SQ, DM], f32, kind="ExternalOutput").ap()

    with ExitStack() as ctx:
        tc = ctx.enter_context(tile.TileContext(nc))

        consts = ctx.enter_context(tc.tile_pool(name="consts", bufs=1))
        w_p = ctx.enter_context(tc.tile_pool(name="w", bufs=1))
        mu_p = ctx.enter_context(tc.tile_pool(name="mu", bufs=1))
        qT_p = ctx.enter_context(tc.tile_pool(name="qT", bufs=1))
        kT_p = ctx.enter_context(tc.tile_pool(name="kT", bufs=1))
        vA_p = ctx.enter_context(tc.tile_pool(name="vA", bufs=1))
        aQ_p = ctx.enter_context(tc.tile_pool(name="aQ", bufs=1))
        stage_p = ctx.enter_context(tc.tile_pool(name="stage", bufs=3))
        sq_p = ctx.enter_context(tc.tile_pool(name="sq", bufs=2))
        nb_p = ctx.enter_context(tc.tile_pool(name="nb", bufs=3))
        stats_p = ctx.enter_context(tc.tile_pool(name="stats", bufs=4))
        pt_p = ctx.enter_context(tc.tile_pool(name="pt", bufs=2))
        lr_p = ctx.enter_context(tc.tile_pool(name="lr", bufs=2))
        outst_p = ctx.enter_context(tc.tile_pool(name="outst", bufs=2))

        psA = ctx.enter_context(tc.tile_pool(name="psA", bufs=2, space="PSUM"))

        # ---- persistent tiles (weights, constants) ----
        # ones stationary for the bias-row matmuls (same for every tile)
        xone = consts.tile([1, 128], bf16, tag="xone")
        nc.vector.memset(xone, 1.0)

        bias_o = consts.tile([128, DM], bf16, tag="bias_o")
        nc.gpsimd.dma_start(
            out=bias_o,
            in_=bass.AP(tensor=bo_d.tensor, offset=bo_d.offset,
                        ap=[[0, 128], bo_d.ap[1]]),
        )

        wts, wbs = {}, {}
        wb_all = consts.tile([1, 3, DM + H], bf16, tag="wb_all")
        for ni, name in enumerate(("q", "k", "v")):
            wt = w_p.tile([128, NIT, DM + H], bf16, tag=f"w_{name}")
            nc.scalar.dma_start(
                out=wt,
                in_=w_d[name][0:DM, :].rearrange("(t p) o -> p t o", p=128),
            )
            wts[name] = wt
            nc.scalar.dma_start(out=wb_all[:, ni, :],
                                in_=w_d[name][DM:DM + 1, :])
            wbs[name] = wb_all[:, ni, :]

        qT = qT_p.tile([128, NIT, SQ], bf16)   # [d-of-headpair, j, q-token]
        kT = kT_p.tile([128, NIT, S], bf16)    # [d-of-headpair, j, k-token]
        vA = vA_p.tile([128, NT_K, H, HD], bf16)  # [k-token, kt, head, d]
        # attention out, query-major: [q-in-chunk, qc, head, d]
        aQ = aQ_p.tile([128, NT_Q, H, HD], bf16)

        ones_c = consts.tile([128, 1], bf16, tag="ones_c")
        nc.vector.memset(ones_c, 1.0)

        magic_t = consts.tile([128, HH], i32, tag="magic")
        nc.vector.memset(magic_t, 0x5f3759df)

        ebias = consts.tile([128, 1], f32, tag="ebias")
        nc.vector.memset(ebias, -4.0)

        # per-head means for every (proj, token-tile): [128, 40, H]
        mu_all = mu_p.tile([128, NT_Q + 2 * NT_K, H], f32)
        MUIDX = {"q": 0, "k": NT_Q, "v": NT_Q + NT_K}
        NTT = {"q": NT_Q, "k": NT_K, "v": NT_K}

        def bcast3(t, n=HD):
            return bass.AP(
                tensor=t.tensor, offset=t.offset,
                ap=[t.ap[0], t.ap[1], [0, n]],
            )

        # xT is only needed until the projections finish: keep it in a nested
        # pool whose space is later reused for Wo and aT
        xT_pool = tc.tile_pool(name="xT", bufs=1)
        xT_p = xT_pool.__enter__()
        xT = xT_p.tile([128, NIT, S], bf16)
        nc.sync.dma_start(
            out=xT, in_=xT_d[0:DM, :].rearrange("(t p) s -> p t s", p=128)
        )

        # ---- phase 0: all per-head means (tiny matmuls; the psM bank is
        # freed before the attention PSUM pools open) ----
        with tc.tile_pool(name="psM", bufs=2, space="PSUM") as psM:
            for name in ("q", "k", "v"):
                wt, wb = wts[name], wbs[name]
                for tt in range(NTT[name]):
                    tsl = slice(tt * 128, (tt + 1) * 128)
                    pm = psM.tile([128, H], f32, tag="psM")
                    for it in range(NIT):
                        nc.tensor.matmul(
                            pm, xT[:, it, tsl], wt[:, it, DM:DM + H],
                            start=(it == 0), stop=False,
                        )
                    nc.tensor.matmul(
                        pm, xone, wb[:, DM:DM + H], start=False, stop=True,
                    )
                    nc.vector.tensor_copy(
                        out=mu_all[:, MUIDX[name] + tt, :], in_=pm)

        psS = ctx.enter_context(tc.tile_pool(name="psS", bufs=2, space="PSUM"))
        psO = ctx.enter_context(tc.tile_pool(name="psO", bufs=1, space="PSUM"))
        psL = ctx.enter_context(tc.tile_pool(name="psL", bufs=1, space="PSUM"))

        # ---- projection main for one (proj, head-half, token-tile) ----
        # norm_eng: "act" while ScalarE is idle (phase 1), "pool" once the
        # attention exps own ScalarE
        def proj_main(name, half, tt, norm_eng):
            wt, wb = wts[name], wbs[name]
            tsl = slice(tt * 128, (tt + 1) * 128)
            csl = slice(half * 512, (half + 1) * 512)
            ps = psA.tile([128, 512], f32, tag="psA")
            for it in range(NIT):
                nc.tensor.matmul(
                    ps, xT[:, it, tsl], wt[:, it, csl],
                    start=(it == 0), stop=False,
                )
            nc.tensor.matmul(ps, xone, wb[:, csl], start=False, stop=True)

            mu = mu_all[:, MUIDX[name] + tt, half * HH:(half + 1) * HH]
            cen = stage_p.tile([128, 512], f32, tag="cen")
            cen3 = cen.rearrange("p (h d) -> p h d", h=HH)
            nc.vector.tensor_sub(
                out=cen3, in0=ps.rearrange("p (h d) -> p h d", h=HH),
                in1=bcast3(mu),
            )
            # square on GPSIMD; per-head sum-of-squares on DVE in bf16
            sqt = sq_p.tile([128, 512], bf16, tag="sq")
            nc.gpsimd.tensor_mul(out=sqt, in0=cen, in1=cen)
            ssq = stats_p.tile([128, HH], bf16, tag="ssq")
            with nc.allow_low_precision(reason="64-term sum; scale noise"):
                nc.vector.tensor_reduce(
                    out=ssq, in_=sqt.rearrange("p (h d) -> p h d", h=HH),
                    axis=mybir.AxisListType.X, op=mybir.AluOpType.add,
                )
            var = stats_p.tile([128, HH], f32, tag="var")
            nc.vector.tensor_scalar(
                out=var, in0=ssq, scalar1=1.0 / HD, scalar2=LN_EPS,
                op0=mybir.AluOpType.mult, op1=mybir.AluOpType.add,
            )
            # rstd = rsqrt(var): magic + 2 Newton iterations (DVE)
            shi = stats_p.tile([128, HH], i32, tag="shi")
            nc.vector.tensor_scalar(
                out=shi, in0=var.bitcast(i32), scalar1=1,
                scalar2=None, op0=mybir.AluOpType.logical_shift_right,
            )
            rstd = stats_p.tile([128, HH], f32, tag="rstd")
            nc.vector.tensor_sub(out=rstd.bitcast(i32), in0=magic_t, in1=shi)
            nt = stats_p.tile([128, HH], f32, tag="nt")
            for _ in range(2):
                nc.vector.tensor_mul(out=nt, in0=rstd, in1=rstd)
                nc.vector.tensor_mul(out=nt, in0=nt, in1=var)
                nc.vector.tensor_scalar(
                    out=nt, in0=nt, scalar1=-0.5, scalar2=1.5,
                    op0=mybir.AluOpType.mult, op1=mybir.AluOpType.add,
                )
                nc.vector.tensor_mul(out=rstd, in0=rstd, in1=nt)

            if name == "v":
                dst3 = vA[:, tt, half * HH:(half + 1) * HH, :]
            else:
                nb = nb_p.tile([128, 512], bf16, tag="nb")
                dst3 = nb.rearrange("p (h d) -> p h d", h=HH)
            if norm_eng == "act":
                for h in range(HH):
                    nc.scalar.activation(
                        out=dst3[:, h, :], in_=cen3[:, h, :],
                        func=mybir.ActivationFunctionType.Copy,
                        scale=rstd[:, h:h + 1],
                    )
            elif norm_eng == "pool":
                nc.gpsimd.tensor_mul(out=dst3, in0=cen3, in1=bcast3(rstd))
            else:
                nc.vector.tensor_mul(out=dst3, in0=cen3, in1=bcast3(rstd))
            if name != "v":
                dst = qT if name == "q" else kT
                nc.sync.dma_start_transpose(
                    dst.bitcast(u16)[:, 4 * half:4 * half + 4, tsl],
                    nb.bitcast(u16),
                )

        # ---- one attention group: (query block, head) ----
        # h = 2j + hh lives on partitions hh*64..hh*64+64 of chunk j
        def attn_group(qb, h):
            j, hh = h // 2, h % 2
            psl = slice(hh * HD, (hh + 1) * HD)
            qsl = slice(qb * QB, (qb + 1) * QB)
            oP = psO.tile([128, NQC, HD], f32, tag="psO", name=f"oP_{qb}_{h}")
            lP = psL.tile([128, NQC], f32, tag="psL", name=f"lP_{qb}_{h}")
            for ktp in range(NT_K // 2):
                sp = psS.tile([128, 2, QB], f32, tag="psS")
                pt2 = pt_p.tile([128, 2, QB], bf16, tag="pt2")
                for i in range(2):
                    ksl = slice((2 * ktp + i) * 128, (2 * ktp + i + 1) * 128)
                    nc.tensor.matmul(
                        sp[:, i, :], kT[psl, j, ksl], qT[psl, j, qsl],
                        start=True, stop=True,
                    )
                nc.scalar.activation(
                    out=pt2, in_=sp,
                    func=mybir.ActivationFunctionType.Exp,
                    scale=1.0 / np.sqrt(HD), bias=ebias[:, 0:1],
                )
                for i in range(2):
                    kt = 2 * ktp + i
                    for c in range(NQC):
                        st = pt2[:, i, c * 128:(c + 1) * 128]
                        first = ktp == 0 and i == 0 and c == 0
                        last = (ktp == NT_K // 2 - 1 and i == 1
                                and c == NQC - 1)
                        nc.tensor.matmul(
                            oP[:, c, :], st, vA[:, kt, h, :],
                            start=first, stop=last,
                            skip_group_check=True,
                        )
                        nc.tensor.matmul(
                            lP[:, c:c + 1], st, ones_c,
                            start=first, stop=last,
                            skip_group_check=True,
                        )
            return oP, lP

        def attn_group_drain(qb, h, oP, lP):
            lr = lr_p.tile([128, NQC], f32, tag="lr")
            nc.vector.reciprocal(out=lr, in_=lP)
            nc.vector.tensor_mul(
                out=aQ[:, qb * NQC:(qb + 1) * NQC, h, :],
                in0=oP, in1=bcast3(lr),
            )

        # ---- phase 1: project head-half 0 ----
        for name in ("q", "k", "v"):
            for tt in range(NTT[name]):
                proj_main(name, 0, tt, norm_eng="pool")

        # ---- phase 2: attention heads 0-7 (x query blocks), interleaved
        # with the head-half-1 projections ----
        proj_h1 = [(name, tt) for name in ("q", "k", "v")
                   for tt in range(NTT[name])]
        groups_h0 = [(qb, h) for qb in range(NQB) for h in range(HH)]
        pi = 0
        for gi, (qb, h) in enumerate(groups_h0):
            oP, lP = attn_group(qb, h)
            attn_group_drain(qb, h, oP, lP)
            take = (len(proj_h1) * (gi + 1)) // len(groups_h0)
            while pi < take:
                proj_main(proj_h1[pi][0], 1, proj_h1[pi][1], norm_eng="pool")
                pi += 1

        # xT's projection inputs are no longer needed: release the pool and
        # load Wo / allocate aT into the freed space
        xT_pool.__exit__(None, None, None)
        late_p = ctx.enter_context(tc.tile_pool(name="late", bufs=1))
        wo = late_p.tile([128, NIT, DM], bf16, tag="wo")
        nc.sync.dma_start(
            out=wo, in_=w_d["o"].rearrange("(t p) o -> p t o", p=128))
        aT = late_p.tile([128, NIT, SQ], bf16, tag="aT")

        def aT_transpose(tq):
            # aQ[:, tq] is [128 q, H, HD] = [128, 1024] bf16
            nc.sync.dma_start_transpose(
                aT.bitcast(u16)[:, :, tq * 128:(tq + 1) * 128],
                aQ[:, tq].rearrange("p h d -> p (h d)").bitcast(u16),
            )

        def outproj_tile(tt):
            tsl = slice(tt * 128, (tt + 1) * 128)
            for oc in range(2):
                ps = psA.tile([128, 512], f32, tag="psA")
                for j in range(NIT):
                    nc.tensor.matmul(
                        ps, aT[:, j, tsl], wo[:, j, oc * 512:(oc + 1) * 512],
                        start=(j == 0), stop=(j == NIT - 1),
                    )
                ot = outst_p.tile([128, 512], f32, tag="outst")
                nc.vector.tensor_add(
                    out=ot, in0=ps, in1=bias_o[:, oc * 512:(oc + 1) * 512])
                nc.sync.dma_start(
                    out=out_d[tsl, oc * 512:(oc + 1) * 512], in_=ot)

        # ---- phase 3: attention heads 8-15; after query block qb finishes
        # for all heads, its aT transposes + out-projection interleave with
        # the next block's attention ----
        for qb in range(NQB):
            ready = list(range((qb - 1) * NQC, qb * NQC)) if qb > 0 else []
            oi = 0
            for gi, h in enumerate(range(HH, H)):
                oP, lP = attn_group(qb, h)
                attn_group_drain(qb, h, oP, lP)
                take = (len(ready) * (gi + 1)) // HH
                while oi < take:
                    tq = ready[oi]
                    aT_transpose(tq)
                    outproj_tile(tq)
                    oi += 1
        for tq in range(NQC, NT_Q):
            aT_transpose(tq)
            outproj_tile(tq)

    nc.compile()
    return nc


_CACHE = {}


def _get_program():
    if "nc" not in _CACHE:
        _CACHE["nc"] = _build_program()
    return _CACHE["nc"]


def _augment(W, b):
    """[W | W@M ; b | b@M] — M averages each head's 64 columns."""
    W = np.asarray(W, dtype=np.float32)
    b = np.asarray(b, dtype=np.float32)
    Wm = W.reshape(DM, H, HD).mean(axis=2)          # [DM, H]
    bm = b.reshape(H, HD).mean(axis=1)              # [H]
    top = np.concatenate([W, Wm], axis=1)           # [DM, DM+H]
    bot = np.concatenate([b, bm])[None, :]          # [1, DM+H]
    return np.ascontiguousarray(
        np.concatenate([top, bot], axis=0).astype(BF16))


def _make_in_maps(x, Wq, bq, Wk, bk, Wv, bv, Wo, bo):
    wq = _augment(Wq, bq)
    wk = _augment(Wk, bk)
    wv = _augment(Wv, bv)
    wo = np.ascontiguousarray(np.asarray(Wo).astype(BF16))
    bo_a = np.ascontiguousarray(np.asarray(bo, dtype=np.float32).reshape(1, DM))
    ones = np.ones((1, S), dtype=np.float32)
    in_maps = []
    for c in range(NCORES):
        b, hf = divmod(c, 2)
        xb = np.asarray(x[b])
        if hf:
            xb = np.concatenate([xb[SQ:], xb[:SQ]], axis=0)
        xt = np.ascontiguousarray(
            np.concatenate([xb.T, ones], axis=0).astype(BF16))
        in_maps.append({
            "xt": xt, "wq": wq, "wk": wk, "wv": wv, "wo": wo, "bo": bo_a,
        })
    return in_maps


def _run(x, Wq, bq, Wk, bk, Wv, bv, Wo, bo, **run_kwargs):
    nc = _get_program()
    in_maps = _make_in_maps(x, Wq, bq, Wk, bk, Wv, bv, Wo, bo)
    res = bass_utils.run_bass_kernel_spmd(
        nc, in_maps, core_ids=list(range(NCORES)), **run_kwargs
    )
    out = np.empty((B, S, DM), dtype=np.float32)
    for c in range(NCORES):
        b, hf = divmod(c, 2)
        out[b, hf * SQ:(hf + 1) * SQ] = res.results[c]["out"]
    return out, res


def kernel(x, Wq, bq, Wk, bk, Wv, bv, Wo, bo):
    out, _ = _run(x, Wq, bq, Wk, bk, Wv, bv, Wo, bo)
    return out


def kernel_profiled(x, Wq, bq, Wk, bk, Wv, bv, Wo, bo):
    return _run(x, Wq, bq, Wk, bk, Wv, bv, Wo, bo, trace=True)
